# revision 12
# baseline (speedup 1.0000x reference)
"""Trainium2 Bass kernel for nn_Conv2d: x[32,128,56,56] * W[256,128,3,3] + b -> [32,256,56,56].

Stride 1, padding 1, dilation 1. Data-parallel over batch across 8 NeuronCores
(4 images per core, no collectives). Per core the conv is one accumulation
group of 9 matmuls per output tile (one per kernel tap):
PSUM[cout_chunk=128, R*56] += matmul(lhsT=Wt[tap][cin, cout_chunk],
rhs=shifted window of the zero-padded input row-block).
Bias is fused into the PSUM->SBUF drain on the scalar engine.

bf16 matmuls (1 cycle/row, fp32 PSUM accumulation, ~2e-3 absmax rel err).
DMA instruction count is kept tiny (the sync engine serializes dma_starts at
~600ns each and the descriptor rings backpressure): x loads are 1 DMA per
image (7 for image 0 so the first matmul group starts early), outputs are
staged in SBUF per (image, cout-chunk) and written with 2 big DMAs each.

Self-contained: hardcodes shapes; host-side pre-pads/retiles x and
pre-transposes W so every device DMA is contiguous per partition.
"""

import numpy as np

B, CIN, H, W_ = 32, 128, 56, 56
COUT, KH, KW = 256, 3, 3
NCORES = 8
BPC = B // NCORES          # images per core
R = 8                      # output rows per tile -> matmul free dim R*56 = 448
NT = H // R                # row tiles per image
HP, WP = H + 2, W_ + 2     # padded 58x58
NCH = COUT // 128          # cout chunks

MM_DTYPE = "bfloat16"

_cache = {}


def _build(mm_dtype_name):
    import concourse.mybir as mybir
    import concourse.tile as tile
    from concourse import bacc

    dt = mybir.dt
    mmdt = getattr(dt, mm_dtype_name)

    nc = bacc.Bacc("TRN2", target_bir_lowering=False, debug=False)

    # x arrives host-pre-padded, one contiguous block per image:
    # [image, cin, row_tile, R+2, 58] (zero border baked in, halo rows
    # duplicated) so a whole image is a single DMA with 8.1KB/partition
    # contiguous descriptors.
    x_d = nc.dram_tensor(
        "x", [BPC, CIN, NT, R + 2, WP], mmdt, kind="ExternalInput"
    )
    # [chunk, cin, tap, cout_slice]: one contiguous DMA per cout chunk
    wt_d = nc.dram_tensor(
        "wt", [NCH, CIN, KH * KW, 128], mmdt, kind="ExternalInput"
    )
    b_d = nc.dram_tensor("bias", [128, NCH], dt.float32, kind="ExternalInput")
    # Output leaves the device in bf16 (halves write-out DMA traffic and the
    # drain tail); the host upcasts back to fp32. Adds <=2^-9 relative error
    # per element on top of the bf16 matmul error -- far inside the gate.
    o_d = nc.dram_tensor("out", [BPC, COUT, H, W_], mmdt, kind="ExternalOutput")

    with tile.TileContext(nc) as tc:
        with (
            tc.tile_pool(name="const", bufs=1) as const_pool,
            tc.tile_pool(name="xin", bufs=1) as xin_pool,
            tc.tile_pool(name="stg", bufs=2) as stg_pool,
            tc.tile_pool(name="psum", bufs=7, space="PSUM") as psum_pool,
            tc.tile_pool(name="warmp", bufs=1, space="PSUM") as warm_pool,
        ):
            # Weights for chunk 0 first: they gate the very first matmul.
            # x loads are issued from the scalar engine's hardware DGE so
            # they stream in parallel with the weight loads on the sync
            # engine's ring (the scalar engine is otherwise idle until the
            # first PSUM drain at ~12us).
            w_t = {}
            for c in range(NCH):
                w_t[c] = const_pool.tile([CIN, KH * KW, 128], mmdt, tag=f"w{c}", name=f"w{c}")
            xt = {}
            for n in range(BPC):
                xt[n] = xin_pool.tile([CIN, NT, R + 2, WP], mmdt, tag=f"x{n}", name=f"x{n}")
            # Critical path for the first matmul group, all on the sync
            # ring in consumption order: x(0,0), then tap 0 of the chunk-0
            # weights (32KB -- unblocks the first LDWEIGHTS ~1.5us before
            # the full chunk could), then taps 1-8 which stream in just
            # ahead of the 192ns/tap consumption rate. The scalar engine's
            # ring carries the rest of image 0 and images 1-3 in parallel.
            nc.sync.dma_start(xt[0][:, 0], x_d[0, :, 0])
            nc.sync.dma_start(w_t[0][:, 0:1], wt_d[0, :, 0:1])
            nc.sync.dma_start(w_t[0][:, 1:], wt_d[0, :, 1:])
            b_t = const_pool.tile([128, NCH], dt.float32)
            nc.sync.dma_start(b_t[:], b_d[:])
            nc.sync.dma_start(w_t[1][:], wt_d[1])
            for ht in range(1, NT):
                nc.scalar.dma_start(xt[0][:, ht], x_d[0, :, ht])
            for n in range(1, BPC):
                nc.scalar.dma_start(xt[n][:], x_d[n])

            # Warm-up matmuls on zeroed scratch: the PE core ramps its clock
            # (p-state) only while executing, so ~3us of throwaway matmuls
            # during the input DMAs lets the real stream start at full rate.
            warm_w = const_pool.tile([CIN, 128], mmdt, name="warm_w")
            warm_x = const_pool.tile([CIN, 448], mmdt, name="warm_x")
            nc.gpsimd.memset(warm_w[:], 0)
            nc.gpsimd.memset(warm_x[:], 0)
            warm_p = warm_pool.tile([128, 448], dt.float32, tag="warm")
            # ~6 x 448 rows at the unramped ~374ns cadence bridges the gap
            # until the first real weights+inputs have landed (~10us) with
            # no PE idle in between, so the clock keeps ramping.
            for _ in range(6):
                nc.tensor.matmul(warm_p[:], warm_w[:], warm_x[:], start=True, stop=True)

            # Loop n -> chunk -> row-tile. The staging tile for (n, c) is
            # flushed to HBM with two DMAs (row-tiles 0-3 and 4-6) so the
            # write-out of one half overlaps the drains of the next.
            for n in range(BPC):
                for c in range(NCH):
                    st = stg_pool.tile([128, NT, R, W_], mmdt, tag="st")
                    last_block = n == BPC - 1 and c == NCH - 1
                    for ht in range(NT):
                        p = psum_pool.tile([128, R, W_], dt.float32, tag="ps")
                        for kh in range(KH):
                            for kw in range(KW):
                                pos = kh * KW + kw
                                nc.tensor.matmul(
                                    p[:],
                                    w_t[c][:, pos],
                                    xt[n][:, ht, kh : kh + R, kw : kw + W_],
                                    start=(pos == 0),
                                    stop=(pos == KH * KW - 1),
                                )
                        if last_block and ht == NT - 1:
                            # Split the very last drain+flush in half-tiles.
                            # The flush DMAs sit pre-decoded at the head of
                            # the otherwise-drained sync queue blocked on the
                            # drain semaphore, so the final (tiny) DMA fires
                            # the moment its 4 rows are drained.
                            for half in range(2):
                                r0, r1 = half * 4, half * 4 + 4
                                nc.scalar.activation(
                                    st[:, ht, r0:r1],
                                    p[:, r0:r1],
                                    mybir.ActivationFunctionType.Identity,
                                    bias=b_t[:, c : c + 1],
                                )
                                nc.sync.dma_start(
                                    o_d[
                                        n,
                                        c * 128 : (c + 1) * 128,
                                        ht * R + r0 : ht * R + r1,
                                        :,
                                    ],
                                    st[:, ht, r0:r1],
                                )
                            continue
                        nc.scalar.activation(
                            st[:, ht],
                            p[:],
                            mybir.ActivationFunctionType.Identity,
                            bias=b_t[:, c : c + 1],
                        )
                        if ht == 3:
                            nc.sync.dma_start(
                                o_d[n, c * 128 : (c + 1) * 128, 0 : 4 * R, :],
                                st[:, 0:4],
                            )
                        elif ht == NT - 1 and not last_block:
                            nc.sync.dma_start(
                                o_d[n, c * 128 : (c + 1) * 128, 4 * R : H, :],
                                st[:, 4:NT],
                            )
                        elif last_block and ht >= 4:
                            nc.sync.dma_start(
                                o_d[n, c * 128 : (c + 1) * 128, ht * R : ht * R + R, :],
                                st[:, ht : ht + 1],
                            )

    nc.compile()
    return nc


def _make_in_maps(x, W, b):
    x = np.asarray(x, dtype=np.float32)
    W = np.asarray(W, dtype=np.float32)
    b = np.asarray(b, dtype=np.float32)

    if MM_DTYPE == "bfloat16":
        import ml_dtypes

        mm_np = ml_dtypes.bfloat16
    else:
        mm_np = np.float32

    # Pre-pad and re-tile x: [B, CIN, 56, 56] -> [B, CIN, NT, R+2, 58] where
    # row-tile ht holds padded rows ht*R .. ht*R+R+1 (zero border baked in,
    # halo rows duplicated across adjacent tiles).
    xpad = np.zeros((B, CIN, HP, WP), dtype=mm_np)
    xpad[:, :, 1 : H + 1, 1 : W_ + 1] = x
    xt = np.empty((B, CIN, NT, R + 2, WP), dtype=mm_np)
    for ht in range(NT):
        xt[:, :, ht] = xpad[:, :, ht * R : ht * R + R + 2, :]

    # [cout, cin, kh, kw] -> [cout_chunk, cin, kh*kw, cout_slice], contiguous
    wt = np.ascontiguousarray(
        W.reshape(NCH, 128, CIN, KH * KW).transpose(0, 2, 3, 1)
    ).astype(mm_np)
    bh = np.ascontiguousarray(b.reshape(NCH, 128).T)

    return [
        {
            "x": xt[core * BPC : (core + 1) * BPC],
            "wt": wt,
            "bias": bh,
        }
        for core in range(NCORES)
    ]


def kernel(x, W, b):
    from concourse.bass_utils import run_bass_kernel_spmd

    if MM_DTYPE not in _cache:
        _cache[MM_DTYPE] = _build(MM_DTYPE)
    nc = _cache[MM_DTYPE]

    in_maps = _make_in_maps(x, W, b)
    try:
        res = run_bass_kernel_spmd(nc, in_maps, list(range(NCORES))).results
    except Exception:
        # A prior session can leave the accelerator in a transient
        # unrecoverable state; one retry after re-init clears it.
        import time

        time.sleep(15)
        res = run_bass_kernel_spmd(nc, in_maps, list(range(NCORES))).results
    return np.concatenate(
        [np.asarray(res[i]["out"]) for i in range(NCORES)], axis=0
    ).astype(np.float32)


# revision 13
# speedup vs baseline: 1.0282x; 1.0282x over previous
"""Trainium2 Bass kernel for nn_Conv2d: x[32,128,56,56] * W[256,128,3,3] + b -> [32,256,56,56].

Stride 1, padding 1, dilation 1. Data-parallel over batch across 8 NeuronCores
(4 images per core, no collectives). Per core the conv is one accumulation
group of 9 matmuls per output tile (one per kernel tap):
PSUM[cout_chunk=128, R*56] += matmul(lhsT=Wt[tap][cin, cout_chunk],
rhs=shifted window of the zero-padded input row-block).
Bias is fused into the PSUM->SBUF drain on the scalar engine.

bf16 matmuls (1 cycle/row, fp32 PSUM accumulation, ~2e-3 absmax rel err).
DMA instruction count is kept tiny (the sync engine serializes dma_starts at
~600ns each and the descriptor rings backpressure): x loads are 1 DMA per
image (7 for image 0 so the first matmul group starts early), outputs are
staged in SBUF per (image, cout-chunk) and written with 2 big DMAs each.

Self-contained: hardcodes shapes; host-side pre-pads/retiles x and
pre-transposes W so every device DMA is contiguous per partition.
"""

import numpy as np

B, CIN, H, W_ = 32, 128, 56, 56
COUT, KH, KW = 256, 3, 3
NCORES = 8
BPC = B // NCORES          # images per core
R = 8                      # output rows per tile -> matmul free dim R*56 = 448
NT = H // R                # row tiles per image
HP, WP = H + 2, W_ + 2     # padded 58x58
NCH = COUT // 128          # cout chunks

MM_DTYPE = "bfloat16"

_cache = {}


def _build(mm_dtype_name):
    import concourse.mybir as mybir
    import concourse.tile as tile
    from concourse import bacc

    dt = mybir.dt
    mmdt = getattr(dt, mm_dtype_name)

    nc = bacc.Bacc("TRN2", target_bir_lowering=False, debug=False)

    # x arrives host-pre-padded, one contiguous block per image:
    # [image, cin, row_tile, R+2, 58] (zero border baked in, halo rows
    # duplicated) so a whole image is a single DMA with 8.1KB/partition
    # contiguous descriptors.
    x_d = nc.dram_tensor(
        "x", [BPC, CIN, NT, R + 2, WP], mmdt, kind="ExternalInput"
    )
    # [chunk, cin, tap, cout_slice]: one contiguous DMA per cout chunk
    wt_d = nc.dram_tensor(
        "wt", [NCH, CIN, KH * KW, 128], mmdt, kind="ExternalInput"
    )
    b_d = nc.dram_tensor("bias", [128, NCH], dt.float32, kind="ExternalInput")
    # Output leaves the device in bf16 (halves write-out DMA traffic and the
    # drain tail); the host upcasts back to fp32. Adds <=2^-9 relative error
    # per element on top of the bf16 matmul error -- far inside the gate.
    o_d = nc.dram_tensor("out", [BPC, COUT, H, W_], mmdt, kind="ExternalOutput")

    with tile.TileContext(nc) as tc:
        with (
            tc.tile_pool(name="const", bufs=1) as const_pool,
            tc.tile_pool(name="xin", bufs=1) as xin_pool,
            tc.tile_pool(name="stg", bufs=2) as stg_pool,
            tc.tile_pool(name="psum", bufs=7, space="PSUM") as psum_pool,
            tc.tile_pool(name="warmp", bufs=1, space="PSUM") as warm_pool,
        ):
            # Weights for chunk 0 first: they gate the very first matmul.
            # x loads are issued from the scalar engine's hardware DGE so
            # they stream in parallel with the weight loads on the sync
            # engine's ring (the scalar engine is otherwise idle until the
            # first PSUM drain at ~12us).
            w_t = {}
            for c in range(NCH):
                w_t[c] = const_pool.tile([CIN, KH * KW, 128], mmdt, tag=f"w{c}", name=f"w{c}")
            xt = {}
            for n in range(BPC):
                xt[n] = xin_pool.tile([CIN, NT, R + 2, WP], mmdt, tag=f"x{n}", name=f"x{n}")
            # Critical path for the first matmul group: chunk-0 weights
            # alone on the sync ring, x(0,0) first on the scalar engine's
            # ring -- the two transfer in parallel and both land ~10us in.
            # Everything else streams behind them. (Splitting w0 so tap 0
            # lands first was tried: the early ring only sustains ~90GB/s,
            # so the remaining taps then arrive slower than the 192ns/tap
            # consumption rate and the stream stutters for ~5us.)
            nc.sync.dma_start(w_t[0][:], wt_d[0])
            b_t = const_pool.tile([128, NCH], dt.float32)
            nc.sync.dma_start(b_t[:], b_d[:])
            for ht in range(NT):
                nc.scalar.dma_start(xt[0][:, ht], x_d[0, :, ht])
            nc.sync.dma_start(w_t[1][:], wt_d[1])
            for n in range(1, BPC):
                nc.scalar.dma_start(xt[n][:], x_d[n])

            # Warm-up matmuls on zeroed scratch: the PE core ramps its clock
            # (p-state) only while executing, so ~3us of throwaway matmuls
            # during the input DMAs lets the real stream start at full rate.
            warm_w = const_pool.tile([CIN, 128], mmdt, name="warm_w")
            warm_x = const_pool.tile([CIN, 448], mmdt, name="warm_x")
            nc.gpsimd.memset(warm_w[:], 0)
            nc.gpsimd.memset(warm_x[:], 0)
            warm_p = warm_pool.tile([128, 448], dt.float32, tag="warm")
            # ~8 x 448 rows at the unramped ~374ns cadence bridges the gap
            # until the first real weights+inputs have landed (~11us) with
            # no PE idle in between, so the clock keeps ramping.
            for _ in range(8):
                nc.tensor.matmul(warm_p[:], warm_w[:], warm_x[:], start=True, stop=True)

            # Loop n -> chunk -> row-tile. The staging tile for (n, c) is
            # flushed to HBM with two DMAs (row-tiles 0-3 and 4-6) so the
            # write-out of one half overlaps the drains of the next.
            for n in range(BPC):
                for c in range(NCH):
                    st = stg_pool.tile([128, NT, R, W_], mmdt, tag="st")
                    last_block = n == BPC - 1 and c == NCH - 1
                    for ht in range(NT):
                        p = psum_pool.tile([128, R, W_], dt.float32, tag="ps")
                        for kh in range(KH):
                            for kw in range(KW):
                                pos = kh * KW + kw
                                nc.tensor.matmul(
                                    p[:],
                                    w_t[c][:, pos],
                                    xt[n][:, ht, kh : kh + R, kw : kw + W_],
                                    start=(pos == 0),
                                    stop=(pos == KH * KW - 1),
                                )
                        nc.scalar.activation(
                            st[:, ht],
                            p[:],
                            mybir.ActivationFunctionType.Identity,
                            bias=b_t[:, c : c + 1],
                        )
                        if ht == 3:
                            nc.sync.dma_start(
                                o_d[n, c * 128 : (c + 1) * 128, 0 : 4 * R, :],
                                st[:, 0:4],
                            )
                        elif ht == NT - 1 and not last_block:
                            nc.sync.dma_start(
                                o_d[n, c * 128 : (c + 1) * 128, 4 * R : H, :],
                                st[:, 4:NT],
                            )
                        elif last_block and ht >= 4:
                            # Flush the final block per row-tile from the
                            # scalar engine right behind its own drain so the
                            # very last DMA is tiny.
                            nc.scalar.dma_start(
                                o_d[n, c * 128 : (c + 1) * 128, ht * R : ht * R + R, :],
                                st[:, ht : ht + 1],
                            )

    nc.compile()
    return nc


def _make_in_maps(x, W, b):
    x = np.asarray(x, dtype=np.float32)
    W = np.asarray(W, dtype=np.float32)
    b = np.asarray(b, dtype=np.float32)

    if MM_DTYPE == "bfloat16":
        import ml_dtypes

        mm_np = ml_dtypes.bfloat16
    else:
        mm_np = np.float32

    # Pre-pad and re-tile x: [B, CIN, 56, 56] -> [B, CIN, NT, R+2, 58] where
    # row-tile ht holds padded rows ht*R .. ht*R+R+1 (zero border baked in,
    # halo rows duplicated across adjacent tiles).
    xpad = np.zeros((B, CIN, HP, WP), dtype=mm_np)
    xpad[:, :, 1 : H + 1, 1 : W_ + 1] = x
    xt = np.empty((B, CIN, NT, R + 2, WP), dtype=mm_np)
    for ht in range(NT):
        xt[:, :, ht] = xpad[:, :, ht * R : ht * R + R + 2, :]

    # [cout, cin, kh, kw] -> [cout_chunk, cin, kh*kw, cout_slice], contiguous
    wt = np.ascontiguousarray(
        W.reshape(NCH, 128, CIN, KH * KW).transpose(0, 2, 3, 1)
    ).astype(mm_np)
    bh = np.ascontiguousarray(b.reshape(NCH, 128).T)

    return [
        {
            "x": xt[core * BPC : (core + 1) * BPC],
            "wt": wt,
            "bias": bh,
        }
        for core in range(NCORES)
    ]


def kernel(x, W, b):
    from concourse.bass_utils import run_bass_kernel_spmd

    if MM_DTYPE not in _cache:
        _cache[MM_DTYPE] = _build(MM_DTYPE)
    nc = _cache[MM_DTYPE]

    in_maps = _make_in_maps(x, W, b)
    try:
        res = run_bass_kernel_spmd(nc, in_maps, list(range(NCORES))).results
    except Exception:
        # A prior session can leave the accelerator in a transient
        # unrecoverable state; one retry after re-init clears it.
        import time

        time.sleep(15)
        res = run_bass_kernel_spmd(nc, in_maps, list(range(NCORES))).results
    return np.concatenate(
        [np.asarray(res[i]["out"]) for i in range(NCORES)], axis=0
    ).astype(np.float32)


# revision 14
# speedup vs baseline: 1.0336x; 1.0052x over previous
"""Trainium2 Bass kernel for nn_Conv2d: x[32,128,56,56] * W[256,128,3,3] + b -> [32,256,56,56].

Stride 1, padding 1, dilation 1. Data-parallel over batch across 8 NeuronCores
(4 images per core, no collectives). Per core the conv is one accumulation
group of 9 matmuls per output tile (one per kernel tap):
PSUM[cout_chunk=128, R*56] += matmul(lhsT=Wt[tap][cin, cout_chunk],
rhs=shifted window of the zero-padded input row-block).
Bias is fused into the PSUM->SBUF drain on the scalar engine.

bf16 matmuls (1 cycle/row, fp32 PSUM accumulation, ~2e-3 absmax rel err).
DMA instruction count is kept tiny (the sync engine serializes dma_starts at
~600ns each and the descriptor rings backpressure): x loads are 1 DMA per
image (7 for image 0 so the first matmul group starts early), outputs are
staged in SBUF per (image, cout-chunk) and written with 2 big DMAs each.

Self-contained: hardcodes shapes; host-side pre-pads/retiles x and
pre-transposes W so every device DMA is contiguous per partition.
"""

import numpy as np

B, CIN, H, W_ = 32, 128, 56, 56
COUT, KH, KW = 256, 3, 3
NCORES = 8
BPC = B // NCORES          # images per core
R = 8                      # output rows per tile -> matmul free dim R*56 = 448
NT = H // R                # row tiles per image
HP, WP = H + 2, W_ + 2     # padded 58x58
NCH = COUT // 128          # cout chunks

MM_DTYPE = "bfloat16"

_cache = {}


def _build(mm_dtype_name):
    import concourse.mybir as mybir
    import concourse.tile as tile
    from concourse import bacc

    dt = mybir.dt
    mmdt = getattr(dt, mm_dtype_name)

    nc = bacc.Bacc("TRN2", target_bir_lowering=False, debug=False)

    # x arrives host-pre-padded, one contiguous block per image:
    # [image, cin, row_tile, R+2, 58] (zero border baked in, halo rows
    # duplicated) so a whole image is a single DMA with 8.1KB/partition
    # contiguous descriptors.
    x_d = nc.dram_tensor(
        "x", [BPC, CIN, NT, R + 2, WP], mmdt, kind="ExternalInput"
    )
    # [chunk, cin, tap, cout_slice]: one contiguous DMA per cout chunk
    wt_d = nc.dram_tensor(
        "wt", [NCH, CIN, KH * KW, 128], mmdt, kind="ExternalInput"
    )
    b_d = nc.dram_tensor("bias", [128, NCH], dt.float32, kind="ExternalInput")
    # Output leaves the device in bf16 (halves write-out DMA traffic and the
    # drain tail); the host upcasts back to fp32. Adds <=2^-9 relative error
    # per element on top of the bf16 matmul error -- far inside the gate.
    o_d = nc.dram_tensor("out", [BPC, COUT, H, W_], mmdt, kind="ExternalOutput")

    with tile.TileContext(nc) as tc:
        with (
            tc.tile_pool(name="const", bufs=1) as const_pool,
            tc.tile_pool(name="xin", bufs=1) as xin_pool,
            tc.tile_pool(name="stg", bufs=2) as stg_pool,
            tc.tile_pool(name="psum", bufs=7, space="PSUM") as psum_pool,
            tc.tile_pool(name="warmp", bufs=1, space="PSUM") as warm_pool,
        ):
            # Weights for chunk 0 first: they gate the very first matmul.
            # x loads are issued from the scalar engine's hardware DGE so
            # they stream in parallel with the weight loads on the sync
            # engine's ring (the scalar engine is otherwise idle until the
            # first PSUM drain at ~12us).
            w_t = {}
            for c in range(NCH):
                w_t[c] = const_pool.tile([CIN, KH * KW, 128], mmdt, tag=f"w{c}", name=f"w{c}")
            xt = {}
            for n in range(BPC):
                xt[n] = xin_pool.tile([CIN, NT, R + 2, WP], mmdt, tag=f"x{n}", name=f"x{n}")
            # Critical path for the first matmul group: chunk-0 weights
            # alone on the sync ring, x(0,0) first on the scalar engine's
            # ring -- the two transfer in parallel and both land ~10us in.
            # Everything else streams behind them. (Splitting w0 so tap 0
            # lands first was tried: the early ring only sustains ~90GB/s,
            # so the remaining taps then arrive slower than the 192ns/tap
            # consumption rate and the stream stutters for ~5us.)
            nc.sync.dma_start(w_t[0][:], wt_d[0])
            b_t = const_pool.tile([128, NCH], dt.float32)
            nc.sync.dma_start(b_t[:], b_d[:])
            for ht in range(NT):
                nc.scalar.dma_start(xt[0][:, ht], x_d[0, :, ht])
            nc.sync.dma_start(w_t[1][:], wt_d[1])
            for n in range(1, BPC):
                nc.scalar.dma_start(xt[n][:], x_d[n])

            # Warm-up matmuls on zeroed scratch: the PE core ramps its clock
            # (p-state) only while executing, so ~3us of throwaway matmuls
            # during the input DMAs lets the real stream start at full rate.
            warm_w = const_pool.tile([CIN, 128], mmdt, name="warm_w")
            warm_x = const_pool.tile([CIN, 448], mmdt, name="warm_x")
            nc.gpsimd.memset(warm_w[:], 0)
            nc.gpsimd.memset(warm_x[:], 0)
            warm_p = warm_pool.tile([128, 448], dt.float32, tag="warm")
            # ~8 x 448 rows at the unramped ~374ns cadence bridges the gap
            # until the first real weights+inputs have landed (~11us) with
            # no PE idle in between, so the clock keeps ramping.
            for _ in range(8):
                nc.tensor.matmul(warm_p[:], warm_w[:], warm_x[:], start=True, stop=True)

            # Loop n -> chunk -> row-tile. The staging tile for (n, c) is
            # flushed to HBM with two DMAs (row-tiles 0-3 and 4-6) so the
            # write-out of one half overlaps the drains of the next.
            for n in range(BPC):
                for c in range(NCH):
                    st = stg_pool.tile([128, NT, R, W_], mmdt, tag="st")
                    last_block = n == BPC - 1 and c == NCH - 1
                    for ht in range(NT):
                        p = psum_pool.tile([128, R, W_], dt.float32, tag="ps")
                        for kh in range(KH):
                            for kw in range(KW):
                                pos = kh * KW + kw
                                nc.tensor.matmul(
                                    p[:],
                                    w_t[c][:, pos],
                                    xt[n][:, ht, kh : kh + R, kw : kw + W_],
                                    start=(pos == 0),
                                    stop=(pos == KH * KW - 1),
                                )
                        if last_block and ht == NT - 1:
                            # The very last drain is on the critical tail:
                            # split it across the scalar and vector engines
                            # so both halves finish in ~0.32us instead of a
                            # 0.67us serial ACTIVATE, and pre-queue the
                            # vector half's flush on the (drained) sync ring.
                            nc.scalar.activation(
                                st[:, ht, 0:4],
                                p[:, 0:4],
                                mybir.ActivationFunctionType.Identity,
                                bias=b_t[:, c : c + 1],
                            )
                            nc.vector.tensor_scalar_add(
                                st[:, ht, 4:R], p[:, 4:R], b_t[:, c : c + 1]
                            )
                            nc.scalar.dma_start(
                                o_d[
                                    n,
                                    c * 128 : (c + 1) * 128,
                                    ht * R : ht * R + 4,
                                    :,
                                ],
                                st[:, ht, 0:4],
                            )
                            nc.sync.dma_start(
                                o_d[
                                    n,
                                    c * 128 : (c + 1) * 128,
                                    ht * R + 4 : ht * R + R,
                                    :,
                                ],
                                st[:, ht, 4:R],
                            )
                            continue
                        nc.scalar.activation(
                            st[:, ht],
                            p[:],
                            mybir.ActivationFunctionType.Identity,
                            bias=b_t[:, c : c + 1],
                        )
                        if ht == 3:
                            nc.sync.dma_start(
                                o_d[n, c * 128 : (c + 1) * 128, 0 : 4 * R, :],
                                st[:, 0:4],
                            )
                        elif ht == NT - 1 and not last_block:
                            nc.sync.dma_start(
                                o_d[n, c * 128 : (c + 1) * 128, 4 * R : H, :],
                                st[:, 4:NT],
                            )
                        elif last_block and ht >= 4:
                            # Flush the final block per row-tile from the
                            # scalar engine right behind its own drain so the
                            # very last DMA is tiny.
                            nc.scalar.dma_start(
                                o_d[n, c * 128 : (c + 1) * 128, ht * R : ht * R + R, :],
                                st[:, ht : ht + 1],
                            )

    nc.compile()
    return nc


def _make_in_maps(x, W, b):
    x = np.asarray(x, dtype=np.float32)
    W = np.asarray(W, dtype=np.float32)
    b = np.asarray(b, dtype=np.float32)

    if MM_DTYPE == "bfloat16":
        import ml_dtypes

        mm_np = ml_dtypes.bfloat16
    else:
        mm_np = np.float32

    # Pre-pad and re-tile x: [B, CIN, 56, 56] -> [B, CIN, NT, R+2, 58] where
    # row-tile ht holds padded rows ht*R .. ht*R+R+1 (zero border baked in,
    # halo rows duplicated across adjacent tiles).
    xpad = np.zeros((B, CIN, HP, WP), dtype=mm_np)
    xpad[:, :, 1 : H + 1, 1 : W_ + 1] = x
    xt = np.empty((B, CIN, NT, R + 2, WP), dtype=mm_np)
    for ht in range(NT):
        xt[:, :, ht] = xpad[:, :, ht * R : ht * R + R + 2, :]

    # [cout, cin, kh, kw] -> [cout_chunk, cin, kh*kw, cout_slice], contiguous
    wt = np.ascontiguousarray(
        W.reshape(NCH, 128, CIN, KH * KW).transpose(0, 2, 3, 1)
    ).astype(mm_np)
    bh = np.ascontiguousarray(b.reshape(NCH, 128).T)

    return [
        {
            "x": xt[core * BPC : (core + 1) * BPC],
            "wt": wt,
            "bias": bh,
        }
        for core in range(NCORES)
    ]


def kernel(x, W, b):
    from concourse.bass_utils import run_bass_kernel_spmd

    if MM_DTYPE not in _cache:
        _cache[MM_DTYPE] = _build(MM_DTYPE)
    nc = _cache[MM_DTYPE]

    in_maps = _make_in_maps(x, W, b)
    try:
        res = run_bass_kernel_spmd(nc, in_maps, list(range(NCORES))).results
    except Exception:
        # A prior session can leave the accelerator in a transient
        # unrecoverable state; one retry after re-init clears it.
        import time

        time.sleep(15)
        res = run_bass_kernel_spmd(nc, in_maps, list(range(NCORES))).results
    return np.concatenate(
        [np.asarray(res[i]["out"]) for i in range(NCORES)], axis=0
    ).astype(np.float32)
